# revision 10
# baseline (speedup 1.0000x reference)
"""Multi-head attention (B=4, N=2048, C=1024, H=16) on 8 TRN2 NeuronCores.

Tensor-parallel over heads: core c owns heads (2c, 2c+1). Each core computes
q/k/v projections for its heads over all tokens, full attention for its heads,
and its heads' slice of the output projection, producing a [B*N, C] f32
partial; the host sums the 8 partials and adds the projection bias.

Layout strategy (all compute in bf16, f32 accumulation):
  - x is passed transposed ([C, B*N] bf16) so QKV matmuls contract over C
    with no on-device transposes.
  - q, k are produced head-dim-major ([2*64, tok]); scores are computed
    TRANSPOSED (s on partitions, q on free dim) so exp comes straight out of
    PSUM; the two heads' K=64 score matmuls are row-packed into one
    [128, 1024] PSUM pair tile.
  - v is produced token-major with a shared ones column ([v_h0 | 1 | v_h1]);
    the M=65 AV matmul then yields both o^T (rows 0-63) and the softmax
    denominator Z (row 64) in one accumulation group.
  - 1/Z scaling uses a DRAM bounce + partition-broadcast DMA load.
  - The output projection consumes o^T directly as lhsT.
"""
import numpy as np
import ml_dtypes

import concourse.bass as bass
import concourse.mybir as mybir
import concourse.tile as tile
from concourse import bacc
from concourse.bass_utils import run_bass_kernel_spmd

B, N, C, H = 4, 2048, 1024, 16
D = C // H  # 64
NCORES = 8
HPC = H // NCORES  # 2 heads per core

BF16 = mybir.dt.bfloat16
F32 = mybir.dt.float32
nbf16 = ml_dtypes.bfloat16


def build(n_batch=B, tok=N):
    """Emit the per-core program. Parameterized so a small config can be
    simulated; the full size is n_batch=4, tok=2048."""
    t_all = n_batch * tok
    n_c = C // 128            # 8 contraction chunks
    n_qk = (tok + 511) // 512  # qk projection N-chunks
    qk_w = min(512, tok)
    n_tt = tok // 128          # token tiles (v, proj)
    n_s = tok // 128           # kv tiles
    n_qc = (tok + 511) // 512  # attention q chunks
    qc_w = min(512, tok)
    scale = float(D) ** -0.5

    nc = bacc.Bacc("TRN2", target_bir_lowering=False, debug=False,
                   num_devices=NCORES)
    xT_d = nc.dram_tensor("xT", [C, t_all], BF16, kind="ExternalInput")
    wqkT_d = nc.dram_tensor("wqkT", [C, 2 * HPC * D], BF16, kind="ExternalInput")
    wvT_d = nc.dram_tensor("wvT", [C, HPC * D], BF16, kind="ExternalInput")
    wpT_d = nc.dram_tensor("wpT", [HPC * D, C], BF16, kind="ExternalInput")
    out_d = nc.dram_tensor("out", [t_all, C], F32, kind="ExternalOutput")
    zout_d = nc.dram_tensor("zout", [n_batch * HPC, tok], F32, kind="ExternalOutput")

    with tile.TileContext(nc) as tc:
        with (
            tc.tile_pool(name="singles", bufs=1) as singles,
            tc.tile_pool(name="xt", bufs=12) as xt_p,
            tc.tile_pool(name="qk", bufs=4) as qk_p,
            tc.tile_pool(name="vp", bufs=24) as v_p,
            tc.tile_pool(name="et", bufs=18) as et_p,
            tc.tile_pool(name="oz", bufs=3) as oz_p,
            tc.tile_pool(name="zb", bufs=2) as zb_p,
            tc.tile_pool(name="osc", bufs=2) as os_p,
            tc.tile_pool(name="ot", bufs=2) as ot_p,
            tc.tile_pool(name="po", bufs=3) as po_p,
            tc.tile_pool(name="ps_mm", bufs=2, space="PSUM") as ps_mm,
            tc.tile_pool(name="ps_sc", bufs=2, space="PSUM") as ps_sc,
            tc.tile_pool(name="ps_av", bufs=2, space="PSUM") as ps_av,
        ):
            wqk_sb = singles.tile([128, n_c, 2 * HPC * D], BF16)
            nc.sync.dma_start(wqk_sb[:], wqkT_d.rearrange("(a p) m -> p a m", p=128))
            wv_sb = singles.tile([128, n_c, HPC * D], BF16)
            nc.sync.dma_start(wv_sb[:], wvT_d.rearrange("(a p) m -> p a m", p=128))
            wp_sb = singles.tile([128, C], BF16)
            nc.sync.dma_start(wp_sb[:], wpT_d[:])

            for b in range(n_batch):
                t0 = b * tok
                # ---- load x^T tiles for this batch ----
                xts = []
                for c in range(n_c):
                    xt = xt_p.tile([128, tok], BF16)
                    nc.sync.dma_start(xt[:], xT_d[c * 128:(c + 1) * 128, t0:t0 + tok])
                    xts.append(xt)

                # ---- qT / kT : [128 = 2 heads x 64, tok] ----
                qkT = []
                for m in range(2):
                    dst = qk_p.tile([128, tok], BF16)
                    for n in range(n_qk):
                        pmm = ps_mm.tile([128, qk_w], F32, tag="mm")
                        for c in range(n_c):
                            nc.tensor.matmul(
                                pmm[:],
                                wqk_sb[:, c, m * 128:(m + 1) * 128],
                                xts[c][:, n * qk_w:(n + 1) * qk_w],
                                start=(c == 0), stop=(c == n_c - 1),
                            )
                        nc.vector.tensor_copy(dst[:, n * qk_w:(n + 1) * qk_w], pmm[:])
                    qkT.append(dst)
                qT, kT = qkT

                # ---- v token-major with shared ones column: [128, 130] ----
                vts = []
                for tt in range(n_tt):
                    pv = ps_mm.tile([128, HPC * D], F32, tag="mm")
                    for c in range(n_c):
                        nc.tensor.matmul(
                            pv[:],
                            xts[c][:, tt * 128:(tt + 1) * 128],
                            wv_sb[:, c, :],
                            start=(c == 0), stop=(c == n_c - 1),
                        )
                    vt = v_p.tile([128, 2 * D + 2], BF16)
                    nc.vector.tensor_copy(vt[:, 0:D], pv[:, 0:D])
                    nc.vector.tensor_copy(vt[:, D + 1:2 * D + 1], pv[:, D:2 * D])
                    nc.gpsimd.memset(vt[:, D:D + 1], 1.0)
                    nc.gpsimd.memset(vt[:, 2 * D + 1:2 * D + 2], 1.0)
                    vts.append(vt)

                # ---- attention (transposed scores; exp from PSUM; M=65 AV) ----
                ozf = [oz_p.tile([D + 1, tok], F32, tag="ozf", name=f"ozf{h}") for h in range(HPC)]
                for qc in range(n_qc):
                    q_sl = slice(qc * qc_w, (qc + 1) * qc_w)
                    ets = []
                    pavs = [ps_av.tile([D + 1, qc_w], F32, tag="av", name=f"pav{h}") for h in range(HPC)]

                    def emit_sc(s):
                        psc = ps_sc.tile([128, 2 * qc_w], F32)
                        for h in range(HPC):
                            nc.tensor.matmul(
                                psc[:, h * qc_w:(h + 1) * qc_w],
                                kT[h * D:(h + 1) * D, s * 128:(s + 1) * 128],
                                qT[h * D:(h + 1) * D, q_sl],
                                start=True, stop=True,
                            )
                        et = et_p.tile([128, 2 * qc_w], BF16)
                        nc.scalar.activation(et[:], psc[:],
                                             mybir.ActivationFunctionType.Exp,
                                             scale=scale)
                        ets.append(et)

                    def emit_av(s):
                        for h in range(HPC):
                            nc.tensor.matmul(
                                pavs[h][:],
                                vts[s][:, h * (D + 1):(h + 1) * (D + 1)],
                                ets[s][:, h * qc_w:(h + 1) * qc_w],
                                start=(s == 0), stop=(s == n_s - 1),
                            )

                    # interleave: sc(s) then av(s-2) keeps PE busy while ACT
                    # works through the exps
                    for s in range(n_s):
                        emit_sc(s)
                        if s >= 2:
                            emit_av(s - 2)
                    for s in range(max(0, n_s - 2), n_s):
                        emit_av(s)
                    for h in range(HPC):
                        nc.vector.tensor_copy(ozf[h][:, q_sl], pavs[h][:])

                # ---- 1/Z scaling (DRAM bounce broadcast) + head merge ----
                oTbig = ot_p.tile([128, tok], BF16)
                for h in range(HPC):
                    zrow = b * HPC + h
                    nc.sync.dma_start(zout_d[zrow:zrow + 1, :], ozf[h][D:D + 1, :])
                    zb = zb_p.tile([D, tok], F32)
                    nc.sync.dma_start(
                        zb[:], zout_d[zrow:zrow + 1, :].to_broadcast((D, tok)))
                    nc.vector.reciprocal(zb[:], zb[:])
                    ost = os_p.tile([D, tok], BF16)
                    nc.vector.tensor_mul(ost[:], ozf[h][0:D, :], zb[:])
                    nc.sync.dma_start(oTbig[h * D:(h + 1) * D, :], ost[:])

                # ---- output projection: out[t, :] partial ----
                for tt in range(n_tt):
                    po = po_p.tile([128, C], F32)
                    for nn in range(C // 512):
                        pp = ps_mm.tile([128, 512], F32, tag="mm")
                        nc.tensor.matmul(
                            pp[:],
                            oTbig[:, tt * 128:(tt + 1) * 128],
                            wp_sb[:, nn * 512:(nn + 1) * 512],
                            start=True, stop=True,
                        )
                        nc.vector.tensor_copy(po[:, nn * 512:(nn + 1) * 512], pp[:])
                    r0 = t0 + tt * 128
                    nc.sync.dma_start(out_d[r0:r0 + 128, :], po[:])

    nc.compile()
    return nc


def prep_in_maps(x, W_qkv, W_proj, n_batch=B, tok=N):
    """Shard + lay out inputs per core (bf16, transposed as the kernel wants)."""
    t_all = n_batch * tok
    x2 = np.ascontiguousarray(
        np.asarray(x, dtype=np.float32).reshape(t_all, C).T).astype(nbf16)
    Wq = np.asarray(W_qkv[0:C], dtype=np.float32)
    Wk = np.asarray(W_qkv[C:2 * C], dtype=np.float32)
    Wv = np.asarray(W_qkv[2 * C:3 * C], dtype=np.float32)
    Wp = np.asarray(W_proj, dtype=np.float32)
    in_maps = []
    for cid in range(NCORES):
        h0, h1 = HPC * cid, HPC * cid + 1
        r0, r1 = slice(h0 * D, (h0 + 1) * D), slice(h1 * D, (h1 + 1) * D)
        wqk = np.concatenate([Wq[r0], Wq[r1], Wk[r0], Wk[r1]], axis=0)
        wv = np.concatenate([Wv[r0], Wv[r1]], axis=0)
        wp = np.concatenate([Wp[:, r0], Wp[:, r1]], axis=1)
        in_maps.append({
            "xT": x2,
            "wqkT": np.ascontiguousarray(wqk.T).astype(nbf16),
            "wvT": np.ascontiguousarray(wv.T).astype(nbf16),
            "wpT": np.ascontiguousarray(wp.T).astype(nbf16),
        })
    return in_maps


_CACHE = {}


def run(x, W_qkv, W_proj, b_proj, trace=False, trace_kwargs=None):
    key = "full"
    if key not in _CACHE:
        _CACHE[key] = build()
    nc = _CACHE[key]
    in_maps = prep_in_maps(x, W_qkv, W_proj)
    res = run_bass_kernel_spmd(
        nc, in_maps, core_ids=list(range(NCORES)), trace=trace,
        **(trace_kwargs or {}))
    acc = res.results[0]["out"].astype(np.float32)
    for i in range(1, NCORES):
        acc += res.results[i]["out"]
    acc += np.asarray(b_proj, dtype=np.float32)[None, :]
    return acc.reshape(B, N, C), res


def kernel(x, W_qkv, W_proj, b_proj):
    out, _ = run(x, W_qkv, W_proj, b_proj)
    return out


# revision 11
# speedup vs baseline: 1.4783x; 1.4783x over previous
"""Multi-head attention (B=4, N=2048, C=1024, H=16) on 8 TRN2 NeuronCores.

Tensor-parallel over heads: core c owns heads (2c, 2c+1). Each core computes
q/k/v projections for its heads over all tokens, full attention for its heads,
and its heads' slice of the output projection, producing a [B*N, C] f32
partial; the host sums the 8 partials and adds the projection bias.

Layout strategy (all compute in bf16, f32 accumulation):
  - x is passed transposed ([C, B*N] bf16) so QKV matmuls contract over C
    with no on-device transposes.
  - q, k are produced head-dim-major ([2*64, tok]); scores are computed
    TRANSPOSED (s on partitions, q on free dim) so exp comes straight out of
    PSUM; the two heads' K=64 score matmuls are row-packed into one
    [128, 1024] PSUM pair tile.
  - v is produced token-major with a shared ones column ([v_h0 | 1 | v_h1]);
    the M=65 AV matmul then yields both o^T (rows 0-63) and the softmax
    denominator Z (row 64) in one accumulation group.
  - 1/Z scaling uses a DRAM bounce + partition-broadcast DMA load.
  - The output projection consumes o^T directly as lhsT.
"""
import numpy as np
import ml_dtypes

import concourse.bass as bass
import concourse.mybir as mybir
import concourse.tile as tile
from concourse import bacc
from concourse.bass_utils import run_bass_kernel_spmd

B, N, C, H = 4, 2048, 1024, 16
D = C // H  # 64
NCORES = 8
HPC = H // NCORES  # 2 heads per core

BF16 = mybir.dt.bfloat16
F32 = mybir.dt.float32
nbf16 = ml_dtypes.bfloat16


def build(n_batch=B, tok=N):
    """Emit the per-core program. Parameterized so a small config can be
    simulated; the full size is n_batch=4, tok=2048."""
    t_all = n_batch * tok
    n_c = C // 128            # 8 contraction chunks
    n_qk = (tok + 511) // 512  # qk projection N-chunks
    qk_w = min(512, tok)
    n_tt = tok // 128          # token tiles (v, proj)
    n_s = tok // 128           # kv tiles
    n_qc = (tok + 511) // 512  # attention q chunks
    qc_w = min(512, tok)
    scale = float(D) ** -0.5

    nc = bacc.Bacc("TRN2", target_bir_lowering=False, debug=False,
                   num_devices=NCORES)
    xT_d = nc.dram_tensor("xT", [C, t_all], BF16, kind="ExternalInput")
    wqkT_d = nc.dram_tensor("wqkT", [C, 2 * HPC * D], BF16, kind="ExternalInput")
    wvT_d = nc.dram_tensor("wvT", [C, HPC * D], BF16, kind="ExternalInput")
    wpT_d = nc.dram_tensor("wpT", [HPC * D, C], BF16, kind="ExternalInput")
    out_d = nc.dram_tensor("out", [t_all, C], F32, kind="ExternalOutput")
    zout_d = nc.dram_tensor("zout", [n_batch * HPC, tok], F32, kind="ExternalOutput")

    with tile.TileContext(nc) as tc:
        with (
            tc.tile_pool(name="singles", bufs=1) as singles,
            tc.tile_pool(name="xt", bufs=12) as xt_p,
            tc.tile_pool(name="qk", bufs=4) as qk_p,
            tc.tile_pool(name="vp", bufs=24) as v_p,
            tc.tile_pool(name="et", bufs=18) as et_p,
            tc.tile_pool(name="oz", bufs=3) as oz_p,
            tc.tile_pool(name="zb", bufs=2) as zb_p,
            tc.tile_pool(name="osc", bufs=2) as os_p,
            tc.tile_pool(name="ot", bufs=2) as ot_p,
            tc.tile_pool(name="po", bufs=3) as po_p,
            tc.tile_pool(name="ps_mm", bufs=2, space="PSUM") as ps_mm,
            tc.tile_pool(name="ps_sc", bufs=2, space="PSUM") as ps_sc,
            tc.tile_pool(name="ps_av", bufs=2, space="PSUM") as ps_av,
        ):
            wqk_sb = singles.tile([128, n_c, 2 * HPC * D], BF16)
            nc.sync.dma_start(wqk_sb[:], wqkT_d.rearrange("(a p) m -> p a m", p=128))
            wv_sb = singles.tile([128, n_c, HPC * D], BF16)
            nc.sync.dma_start(wv_sb[:], wvT_d.rearrange("(a p) m -> p a m", p=128))
            wp_sb = singles.tile([128, C], BF16)
            nc.sync.dma_start(wp_sb[:], wpT_d[:])

            def phase_qkv(b):
                """Load x^T, produce qT/kT (head-dim-major) and v (+ones)."""
                t0 = b * tok
                xts = []
                for c in range(n_c):
                    xt = xt_p.tile([128, tok], BF16)
                    nc.sync.dma_start(xt[:], xT_d[c * 128:(c + 1) * 128, t0:t0 + tok])
                    xts.append(xt)

                qkT = []
                for m in range(2):
                    dst = qk_p.tile([128, tok], BF16)
                    for n in range(n_qk):
                        pmm = ps_mm.tile([128, qk_w], F32, tag="mm")
                        for c in range(n_c):
                            nc.tensor.matmul(
                                pmm[:],
                                wqk_sb[:, c, m * 128:(m + 1) * 128],
                                xts[c][:, n * qk_w:(n + 1) * qk_w],
                                start=(c == 0), stop=(c == n_c - 1),
                            )
                        nc.vector.tensor_copy(dst[:, n * qk_w:(n + 1) * qk_w], pmm[:])
                    qkT.append(dst)

                vts = []
                for tt in range(n_tt):
                    pv = ps_mm.tile([128, HPC * D], F32, tag="mm")
                    for c in range(n_c):
                        nc.tensor.matmul(
                            pv[:],
                            xts[c][:, tt * 128:(tt + 1) * 128],
                            wv_sb[:, c, :],
                            start=(c == 0), stop=(c == n_c - 1),
                        )
                    vt = v_p.tile([128, 2 * D + 2], BF16)
                    nc.vector.tensor_copy(vt[:, 0:D], pv[:, 0:D])
                    nc.vector.tensor_copy(vt[:, D + 1:2 * D + 1], pv[:, D:2 * D])
                    nc.gpsimd.memset(vt[:, D:D + 1], 1.0)
                    nc.gpsimd.memset(vt[:, 2 * D + 1:2 * D + 2], 1.0)
                    vts.append(vt)
                return {"qT": qkT[0], "kT": qkT[1], "vts": vts}

            def phase_attn(b, st):
                """Transposed scores -> exp -> M=65 AV; fills st["ozf"]."""
                qT, kT, vts = st["qT"], st["kT"], st["vts"]
                ozf = [oz_p.tile([D + 1, tok], F32, tag="ozf", name=f"ozf{h}")
                       for h in range(HPC)]
                for qc in range(n_qc):
                    q_sl = slice(qc * qc_w, (qc + 1) * qc_w)
                    ets = []
                    pavs = [ps_av.tile([D + 1, qc_w], F32, tag="av", name=f"pav{h}")
                            for h in range(HPC)]

                    def emit_sc(s):
                        psc = ps_sc.tile([128, 2 * qc_w], F32)
                        for h in range(HPC):
                            nc.tensor.matmul(
                                psc[:, h * qc_w:(h + 1) * qc_w],
                                kT[h * D:(h + 1) * D, s * 128:(s + 1) * 128],
                                qT[h * D:(h + 1) * D, q_sl],
                                start=True, stop=True,
                            )
                        et = et_p.tile([128, 2 * qc_w], BF16)
                        nc.scalar.activation(et[:], psc[:],
                                             mybir.ActivationFunctionType.Exp,
                                             scale=scale)
                        ets.append(et)

                    def emit_av(s):
                        for h in range(HPC):
                            nc.tensor.matmul(
                                pavs[h][:],
                                vts[s][:, h * (D + 1):(h + 1) * (D + 1)],
                                ets[s][:, h * qc_w:(h + 1) * qc_w],
                                start=(s == 0), stop=(s == n_s - 1),
                            )

                    # interleave: sc(s) then av(s-2) keeps PE busy while ACT
                    # works through the exps
                    for s in range(n_s):
                        emit_sc(s)
                        if s >= 2:
                            emit_av(s - 2)
                    for s in range(max(0, n_s - 2), n_s):
                        emit_av(s)
                    for h in range(HPC):
                        nc.vector.tensor_copy(ozf[h][:, q_sl], pavs[h][:])
                st["ozf"] = ozf

            def phase_zproj(b, st):
                """1/Z scaling (DRAM bounce broadcast), head merge, projection."""
                t0 = b * tok
                ozf = st["ozf"]
                oTbig = ot_p.tile([128, tok], BF16)
                for h in range(HPC):
                    zrow = b * HPC + h
                    nc.sync.dma_start(zout_d[zrow:zrow + 1, :], ozf[h][D:D + 1, :])
                    zb = zb_p.tile([D, tok], F32)
                    nc.sync.dma_start(
                        zb[:], zout_d[zrow:zrow + 1, :].to_broadcast((D, tok)))
                    nc.vector.reciprocal_approx_fast(zb[:], zb[:])
                    ost = os_p.tile([D, tok], BF16)
                    nc.vector.tensor_mul(ost[:], ozf[h][0:D, :], zb[:])
                    nc.sync.dma_start(oTbig[h * D:(h + 1) * D, :], ost[:])

                for tt in range(n_tt):
                    po = po_p.tile([128, C], F32)
                    for nn in range(C // 512):
                        pp = ps_mm.tile([128, 512], F32, tag="mm")
                        nc.tensor.matmul(
                            pp[:],
                            oTbig[:, tt * 128:(tt + 1) * 128],
                            wp_sb[:, nn * 512:(nn + 1) * 512],
                            start=True, stop=True,
                        )
                        nc.vector.tensor_copy(po[:, nn * 512:(nn + 1) * 512], pp[:])
                    r0 = t0 + tt * 128
                    nc.sync.dma_start(out_d[r0:r0 + 128, :], po[:])

            # software pipeline: batch b's Z-chain + projection are emitted
            # after batch b+1's QKV so PE never drains while the Z bounce and
            # scaling run on DMA/DVE
            prev = None
            for b in range(n_batch):
                st = phase_qkv(b)
                if prev is not None:
                    phase_zproj(b - 1, prev)
                phase_attn(b, st)
                prev = st
            phase_zproj(n_batch - 1, prev)

    nc.compile()
    return nc


def prep_in_maps(x, W_qkv, W_proj, n_batch=B, tok=N):
    """Shard + lay out inputs per core (bf16, transposed as the kernel wants)."""
    t_all = n_batch * tok
    x2 = np.ascontiguousarray(
        np.asarray(x, dtype=np.float32).reshape(t_all, C).T).astype(nbf16)
    Wq = np.asarray(W_qkv[0:C], dtype=np.float32)
    Wk = np.asarray(W_qkv[C:2 * C], dtype=np.float32)
    Wv = np.asarray(W_qkv[2 * C:3 * C], dtype=np.float32)
    Wp = np.asarray(W_proj, dtype=np.float32)
    in_maps = []
    for cid in range(NCORES):
        h0, h1 = HPC * cid, HPC * cid + 1
        r0, r1 = slice(h0 * D, (h0 + 1) * D), slice(h1 * D, (h1 + 1) * D)
        wqk = np.concatenate([Wq[r0], Wq[r1], Wk[r0], Wk[r1]], axis=0)
        wv = np.concatenate([Wv[r0], Wv[r1]], axis=0)
        wp = np.concatenate([Wp[:, r0], Wp[:, r1]], axis=1)
        in_maps.append({
            "xT": x2,
            "wqkT": np.ascontiguousarray(wqk.T).astype(nbf16),
            "wvT": np.ascontiguousarray(wv.T).astype(nbf16),
            "wpT": np.ascontiguousarray(wp.T).astype(nbf16),
        })
    return in_maps


_CACHE = {}


def run(x, W_qkv, W_proj, b_proj, trace=False, trace_kwargs=None):
    key = "full"
    if key not in _CACHE:
        _CACHE[key] = build()
    nc = _CACHE[key]
    in_maps = prep_in_maps(x, W_qkv, W_proj)
    res = run_bass_kernel_spmd(
        nc, in_maps, core_ids=list(range(NCORES)), trace=trace,
        **(trace_kwargs or {}))
    acc = res.results[0]["out"].astype(np.float32)
    for i in range(1, NCORES):
        acc += res.results[i]["out"]
    acc += np.asarray(b_proj, dtype=np.float32)[None, :]
    return acc.reshape(B, N, C), res


def kernel(x, W_qkv, W_proj, b_proj):
    out, _ = run(x, W_qkv, W_proj, b_proj)
    return out
